# revision 26
# baseline (speedup 1.0000x reference)
"""Trainium2 Bass kernel: x + s -> LayerNorm(W) -> 2x2x2 avgpool -> exact GELU.

Input  x: (32, 32, 16, 32, 64) f32, sum_weight (1,), gamma (64,), beta (64,)
Output:   (32, 32, 8, 16, 32) f32

Math:
  LN is shift-invariant, so sum_weight cancels exactly.
  pooled[q, w'] = sum_{r in quad} y_r (ga x_e + go x_o)[w'] - gw''[w'] mq[q] + bw[w']
    y_r   = rho_r / 8 = rsqrt(64 var_r)   (rsqrt via bit-magic + 1 Newton step)
    mq[q] = sum_{r in quad} r1_r y_r,  gw'' = (ga+go)/64,  bw = (be+bo)/2
  out = Gelu(pooled)

Layout: data-parallel over batch N (4 per core x 8 cores). Partitions = the
128 (n, c) pairs. Host pre-permutes d/h/w into even|odd halves and converts
to bf16, so every bulk op is a contiguous-half TENSOR_TENSOR that hits the
DVE 2x bf16 fast path (0.54 ns/elem measured on HW). Per-row stats come
from a 6-level pairwise add tree over a flat tile holding [x | x^2]; Square
and Gelu share one ACT table (one table load total). Everything else runs
on DVE: GPSIMD TT is ~2.6 ns/elem AND stalls concurrent DVE ops via shared
SBUF ports, so offloading there is net-negative. Chunks are variable-size
(small first/last) to cut pipeline fill and drain; issue order interleaves
A-phase of chunk k+2 with B-phase of chunk k.
"""

import numpy as np

import concourse.bacc as bacc
import concourse.bass as bass
import concourse.tile as tile
from concourse import mybir
from concourse.bass_utils import run_bass_kernel_spmd

P = 128
N, C, D, H, W = 32, 32, 16, 32, 64
NCORES = 8
NPER = N // NCORES
EPS = 1e-5
F32 = mybir.dt.float32
BF16 = mybir.dt.bfloat16
U32 = mybir.dt.uint32
NP_BF16 = mybir.dt.np(BF16)

# variable-size chunks (in d-planes): small first/last to cut pipeline
# fill and drain; middles large for low instruction overhead
CHUNKS = [(0, 2), (2, 4), (6, 4), (10, 4), (14, 2)]
HW2 = (H // 2) * 32   # 512: h' x w' per d' slot
MAGIC = 0x5F3759DF

AF = mybir.ActivationFunctionType
OP = mybir.AluOpType


def _kernel_body(ctx, tc: tile.TileContext, out_ap: bass.AP, xs: bass.AP, cons: bass.AP):
    nc = tc.nc

    singles = ctx.enter_context(tc.tile_pool(name="singles", bufs=1))
    xpool = ctx.enter_context(tc.tile_pool(name="xpool", bufs=2))
    tpool = ctx.enter_context(tc.tile_pool(name="tpool", bufs=1))
    t6pool = ctx.enter_context(tc.tile_pool(name="t6pool", bufs=3))
    gpool = ctx.enter_context(tc.tile_pool(name="gpool", bufs=2))
    spool = ctx.enter_context(tc.tile_pool(name="spool", bufs=2))
    small = ctx.enter_context(tc.tile_pool(name="small", bufs=2))
    opool = ctx.enter_context(tc.tile_pool(name="opool", bufs=2))

    # constants (bf16), broadcast to all partitions
    grep_t = singles.tile([P, 64], BF16)   # [ga(32) | go(32)], raw gamma deint
    nc.sync.dma_start(out=grep_t[:], in_=cons[0:1, :].to_broadcast((P, 64)))
    gwbw_t = singles.tile([P, 64], BF16)   # [gw''(32) | bw(32)]
    nc.sync.dma_start(out=gwbw_t[:], in_=cons[1:2, :].to_broadcast((P, 64)))
    gw_t = gwbw_t[:, 0:32]
    bw_t = gwbw_t[:, 32:64]
    magic_t = singles.tile([P, 1], U32)
    nc.vector.memset(magic_t[:], MAGIC)

    xsf = xs.rearrange("p d h w -> p (d h w)")
    outf = out_ap.rearrange("p d h w -> p d (h w)")  # d' dim = 8 = NCHUNK*2

    state = {}

    NDMAX = 4
    RMAX = NDMAX * H
    CMAX = RMAX * W
    NQMAX = (NDMAX // 2) * (H // 2)

    def emit_A(k):
        d0, nd = CHUNKS[k]
        ROWS = nd * H
        CH = ROWS * W
        DC = nd
        NQ = (DC // 2) * (H // 2)
        # DMA + ACT square + DVE: gamma/wpool/tree/newton; GP: smalls
        tXf = xpool.tile([P, 2 * CMAX], BF16, tag="tX")
        tx0 = tXf[:, 0:CH]
        tx1 = tXf[:, CH : 2 * CH]
        nc.sync.dma_start(out=tx0, in_=xsf[:, d0 * H * W : (d0 + nd) * H * W])
        nc.scalar.activation(tx1, tx0, AF.Square)

        # g = x * gamma_rep ; s0 = g_lo + g_hi   (DVE bf16 2x)
        gf = gpool.tile([P, RMAX, W], BF16, tag="g")
        g = gf[:, 0:ROWS, :]
        nc.vector.tensor_tensor(
            out=g,
            in0=tx0.rearrange("p (r w) -> p r w", w=W),
            in1=grep_t[:].unsqueeze(1).to_broadcast((P, ROWS, W)),
            op=OP.mult,
        )
        s0f = spool.tile([P, RMAX, 32], BF16, tag="s0")
        s0 = s0f[:, 0:ROWS, :]
        nc.vector.tensor_tensor(
            out=s0, in0=g[:, :, 0:32], in1=g[:, :, 32:64], op=OP.add,
        )

        # stats tree: 6 levels of pairwise adds over [x | x^2]
        t_in = tXf[:, 0 : 2 * CH].rearrange("p (q w) -> p q w", w=W)
        for li, wd in enumerate((32, 16, 8, 4, 2, 1)):
            pool_l = t6pool if wd == 1 else tpool
            tlf = pool_l.tile([P, 2 * RMAX, wd], F32 if wd == 1 else BF16,
                              tag=f"tree{li}")
            tl = tlf[:, 0 : 2 * ROWS, :]
            nc.vector.tensor_tensor(
                out=tl, in0=t_in[:, :, 0:wd], in1=t_in[:, :, wd : 2 * wd],
                op=OP.add,
            )
            t_in = tl
        r1 = t_in[:, 0:ROWS, 0]           # [P, ROWS] f32: sum x
        r2 = t_in[:, ROWS : 2 * ROWS, 0]  # [P, ROWS] f32: sum x^2

        def sm(tag, dt=F32):
            return small.tile([P, RMAX], dt, tag=tag, name=tag)[:, 0:ROWS]

        # rsqrt(64*var) via bit magic + 1 Newton step (DVE smalls, f32)
        msq = sm("msq")
        nc.vector.tensor_tensor(out=msq, in0=r1, in1=r1, op=OP.mult)
        wv2 = sm("wv2")
        nc.vector.scalar_tensor_tensor(
            out=wv2, in0=msq, scalar=-1.0 / W, in1=r2,
            op0=OP.mult, op1=OP.add,
        )
        yi = sm("yi", U32)
        nc.vector.tensor_scalar(
            out=yi, in0=wv2.bitcast(U32), scalar1=1, scalar2=None,
            op0=OP.logical_shift_right,
        )
        y0 = sm("y0", U32)
        nc.vector.tensor_tensor(
            out=y0, in0=magic_t[:].to_broadcast((P, ROWS)), in1=yi,
            op=OP.subtract,
        )
        ys = y0.bitcast(F32)
        a = sm("nta")
        nc.vector.tensor_tensor(out=a, in0=ys, in1=ys, op=OP.mult)
        b = sm("ntb")
        nc.vector.scalar_tensor_tensor(
            out=b, in0=a, scalar=-0.5, in1=wv2, op0=OP.mult, op1=OP.mult
        )
        y = sm("nty")
        nc.vector.scalar_tensor_tensor(
            out=y, in0=b, scalar=1.5, in1=ys, op0=OP.add, op1=OP.mult
        )

        # sr = s0 * y. For big mid-stream chunks, ACT (idle) materializes the
        # y broadcast into bf16 so the multiply runs as a packed 2x TT; the
        # small first/last chunks take the direct 1x path to keep the ACT
        # queue from head-blocking at startup/drain.
        srf = spool.tile([P, RMAX, 32], BF16, tag="sr")
        sr = srf[:, 0:ROWS, :]
        if k < len(CHUNKS) - 1:
            yrep = gpool.tile([P, RMAX, W], BF16, tag="g", name="yrep")
            nc.scalar.activation(
                yrep[:, 0:ROWS, 0:32],
                y.unsqueeze(2).to_broadcast((P, ROWS, 32)),
                AF.Copy,
            )
            nc.vector.tensor_tensor(
                out=sr, in0=s0, in1=yrep[:, 0:ROWS, 0:32], op=OP.mult,
            )
        else:
            nc.vector.tensor_tensor(
                out=sr, in0=s0,
                in1=y.unsqueeze(2).to_broadcast((P, ROWS, 32)), op=OP.mult,
            )
        sr4 = sr.rearrange("p (s h) w -> p s (h w)", s=DC)
        xdf = spool.tile([P, NDMAX // 2, H * 32], BF16, tag="xd")
        xd = xdf[:, 0 : DC // 2, :]
        nc.vector.tensor_tensor(
            out=xd, in0=sr4[:, 0 : DC // 2, :], in1=sr4[:, DC // 2 : DC, :],
            op=OP.add,
        )
        xhf = spool.tile([P, NDMAX // 2, HW2], BF16, tag="xh")
        xh = xhf[:, 0 : DC // 2, :]
        nc.vector.tensor_tensor(
            out=xh, in0=xd[:, :, 0:HW2], in1=xd[:, :, HW2 : 2 * HW2],
            op=OP.add,
        )
        # GPSIMD small burst: mrs/m1/mq/corr
        mrs = sm("mrs")
        nc.gpsimd.tensor_tensor(out=mrs, in0=r1, in1=y, op=OP.mult)
        mrs4 = mrs.rearrange("p (s h) -> p s h", s=DC)
        m1 = small.tile([P, NDMAX // 2, H], F32, tag="m1", name="m1")[:, 0 : DC // 2, :]
        nc.gpsimd.tensor_tensor(
            out=m1, in0=mrs4[:, 0 : DC // 2, :], in1=mrs4[:, DC // 2 : DC, :],
            op=OP.add,
        )
        mq = small.tile([P, NDMAX // 2, H // 2], F32, tag="mq", name="mq")[:, 0 : DC // 2, :]
        nc.gpsimd.tensor_tensor(
            out=mq, in0=m1[:, :, 0 : H // 2], in1=m1[:, :, H // 2 : H],
            op=OP.add,
        )
        corr = spool.tile([P, NQMAX, 32], BF16, tag="corr", name="corr")[:, 0:NQ, :]
        mq_b = mq.rearrange("p s h -> p (s h)").unsqueeze(2).to_broadcast((P, NQ, 32))
        if k < len(CHUNKS) - 1:
            mqrep = opool.tile([P, NQMAX * 32], BF16, tag="pre", name="mqrep")
            mqr = mqrep[:, 0 : NQ * 32].rearrange("p (a b) -> p a b", b=32)
            nc.scalar.activation(mqr, mq_b, AF.Copy)
            nc.vector.tensor_tensor(
                out=corr, in0=gw_t.unsqueeze(1).to_broadcast((P, NQ, 32)),
                in1=mqr, op=OP.mult,
            )
        else:
            nc.vector.tensor_tensor(
                out=corr, in0=gw_t.unsqueeze(1).to_broadcast((P, NQ, 32)),
                in1=mq_b, op=OP.mult,
            )
        state[k] = (xh, corr, d0, nd)

    def emit_B(k):
        xh, corr, d0, nd = state.pop(k)
        NQ = (nd // 2) * (H // 2)
        pre = opool.tile([P, NQMAX * 32], BF16, tag="pre", name="pre")[:, 0 : NQ * 32]
        nc.vector.tensor_tensor(
            out=pre,
            in0=xh.rearrange("p a b -> p (a b)"),
            in1=corr.rearrange("p a b -> p (a b)"),
            op=OP.subtract,
        )
        pre2 = opool.tile([P, NQMAX, 32], BF16, tag="pre2", name="pre2")[:, 0:NQ, :]
        nc.vector.tensor_tensor(
            out=pre2,
            in0=pre.rearrange("p (a b) -> p a b", b=32),
            in1=bw_t.unsqueeze(1).to_broadcast((P, NQ, 32)),
            op=OP.add,
        )
        res = opool.tile([P, NQMAX * 32], BF16, tag="res", name="res")[:, 0 : NQ * 32]
        nc.scalar.activation(
            res, pre2.rearrange("p a b -> p (a b)"), AF.Gelu
        )
        nc.sync.dma_start(
            out=outf[:, d0 // 2 : d0 // 2 + nd // 2, :],
            in_=res.rearrange("p (a b) -> p a b", b=HW2),
        )

    # software pipeline: A(0) A(1) B(0) A(2) B(1) ...
    NC_ = len(CHUNKS)
    emit_A(0)
    emit_A(1)
    for k in range(NC_):
        if k + 2 < NC_:
            emit_A(k + 2)
        emit_B(k)


_CACHE: dict = {}


def _get_compiled():
    if "nc" not in _CACHE:
        nc = bacc.Bacc("TRN2", target_bir_lowering=False, debug=False)
        xs = nc.dram_tensor("xs", [P, D, H, W], BF16, kind="ExternalInput").ap()
        cons = nc.dram_tensor("cons", [2, 64], BF16, kind="ExternalInput").ap()
        out = nc.dram_tensor(
            "out", [P, D // 2, H // 2, W // 2], BF16, kind="ExternalOutput"
        ).ap()
        from contextlib import ExitStack

        with tile.TileContext(nc) as tc, ExitStack() as ctx:
            _kernel_body(ctx, tc, out, xs, cons)
        nc.compile()
        _CACHE["nc"] = nc
    return _CACHE["nc"]


# host-side index permutations: even|odd halves for d (per chunk), h, w
_DORD = np.concatenate([
    np.concatenate([np.arange(d0, d0 + nd, 2), np.arange(d0 + 1, d0 + nd, 2)])
    for d0, nd in CHUNKS
])
_HORD = np.concatenate([np.arange(0, H, 2), np.arange(1, H, 2)])
_WORD = np.concatenate([np.arange(0, W, 2), np.arange(1, W, 2)])


def _make_cons(gamma: np.ndarray, beta: np.ndarray) -> np.ndarray:
    ga = gamma[0::2].astype(np.float64)
    go = gamma[1::2].astype(np.float64)
    grep = np.concatenate([ga, go])                      # raw, deinterleaved
    gw = (ga + go) / float(W)                            # gw'' = (ga+go)/64
    bw = 0.5 * (beta[0::2] + beta[1::2]).astype(np.float64)
    row1 = np.concatenate([gw, bw])
    return np.stack([grep, row1]).astype(NP_BF16)


def kernel(x, sum_weight, gamma, beta, trace=False):
    del sum_weight  # cancels exactly in LayerNorm (shift invariance)
    nc = _get_compiled()
    x = np.asarray(x)
    # permute d/h/w into even|odd halves, cast bf16
    xp = x[:, :, _DORD][:, :, :, _HORD][:, :, :, :, _WORD].astype(NP_BF16)
    cons = _make_cons(np.asarray(gamma), np.asarray(beta))
    in_maps = []
    for core in range(NCORES):
        shard = np.ascontiguousarray(
            xp[core * NPER : (core + 1) * NPER].reshape(P, D, H, W)
        )
        in_maps.append({"xs": shard, "cons": cons})
    res = run_bass_kernel_spmd(nc, in_maps, core_ids=list(range(NCORES)), trace=trace)
    out = np.concatenate(
        [
            res.results[i]["out"]
            .reshape(NPER, C, D // 2, H // 2, W // 2)
            .astype(np.float32)
            for i in range(NCORES)
        ],
        axis=0,
    )
    if trace:
        return out, res
    return out


if __name__ == "__main__":
    rng = np.random.default_rng(0)
    x = rng.standard_normal((N, C, D, H, W), dtype=np.float32)
    sw = rng.standard_normal((1,)).astype(np.float32)
    gamma = rng.random((W,), dtype=np.float32)
    beta = rng.standard_normal((W,)).astype(np.float32)
    y = kernel(x, sw, gamma, beta)
    print(y.shape, y.dtype)


# revision 27
# speedup vs baseline: 1.0221x; 1.0221x over previous
"""Trainium2 Bass kernel: x + s -> LayerNorm(W) -> 2x2x2 avgpool -> exact GELU.

Input  x: (32, 32, 16, 32, 64) f32, sum_weight (1,), gamma (64,), beta (64,)
Output:   (32, 32, 8, 16, 32) f32

Math:
  LN is shift-invariant, so sum_weight cancels exactly.
  pooled[q, w'] = sum_{r in quad} y_r (ga x_e + go x_o)[w'] - gw''[w'] mq[q] + bw[w']
    y_r   = rho_r / 8 = rsqrt(64 var_r)   (rsqrt via bit-magic + 1 Newton step)
    mq[q] = sum_{r in quad} r1_r y_r,  gw'' = (ga+go)/64,  bw = (be+bo)/2
  out = Gelu(pooled)

Layout: data-parallel over batch N (4 per core x 8 cores). Partitions = the
128 (n, c) pairs. Host pre-permutes d/h/w into even|odd halves and converts
to bf16, so every bulk op is a contiguous-half TENSOR_TENSOR that hits the
DVE 2x bf16 fast path (0.54 ns/elem measured on HW). Per-row stats come
from a 6-level pairwise add tree over a flat tile holding [x | x^2]; Square
and Gelu share one ACT table (one table load total). Everything else runs
on DVE: GPSIMD TT is ~2.6 ns/elem AND stalls concurrent DVE ops via shared
SBUF ports, so offloading there is net-negative. Chunks are variable-size
(small first/last) to cut pipeline fill and drain; issue order interleaves
A-phase of chunk k+2 with B-phase of chunk k.
"""

import numpy as np

import concourse.bacc as bacc
import concourse.bass as bass
import concourse.tile as tile
from concourse import mybir
from concourse.bass_utils import run_bass_kernel_spmd

P = 128
N, C, D, H, W = 32, 32, 16, 32, 64
NCORES = 8
NPER = N // NCORES
EPS = 1e-5
F32 = mybir.dt.float32
BF16 = mybir.dt.bfloat16
U32 = mybir.dt.uint32
NP_BF16 = mybir.dt.np(BF16)

# variable-size chunks (in d-planes): small first/last to cut pipeline
# fill and drain; middles large for low instruction overhead
CHUNKS = [(0, 2), (2, 4), (6, 4), (10, 4), (14, 2)]
HW2 = (H // 2) * 32   # 512: h' x w' per d' slot
MAGIC = 0x5F3759DF

AF = mybir.ActivationFunctionType
OP = mybir.AluOpType


def _kernel_body(ctx, tc: tile.TileContext, out_ap: bass.AP, xs: bass.AP, cons: bass.AP):
    nc = tc.nc

    singles = ctx.enter_context(tc.tile_pool(name="singles", bufs=1))
    xpool = ctx.enter_context(tc.tile_pool(name="xpool", bufs=2))
    tpool = ctx.enter_context(tc.tile_pool(name="tpool", bufs=1))
    t6pool = ctx.enter_context(tc.tile_pool(name="t6pool", bufs=3))
    gpool = ctx.enter_context(tc.tile_pool(name="gpool", bufs=2))
    spool = ctx.enter_context(tc.tile_pool(name="spool", bufs=2))
    small = ctx.enter_context(tc.tile_pool(name="small", bufs=2))
    opool = ctx.enter_context(tc.tile_pool(name="opool", bufs=2))

    # constants (bf16), broadcast to all partitions
    grep_t = singles.tile([P, 64], BF16)   # [ga(32) | go(32)], raw gamma deint
    nc.sync.dma_start(out=grep_t[:], in_=cons[0:1, :].to_broadcast((P, 64)))
    gwbw_t = singles.tile([P, 64], BF16)   # [gw''(32) | bw(32)]
    nc.sync.dma_start(out=gwbw_t[:], in_=cons[1:2, :].to_broadcast((P, 64)))
    gw_t = gwbw_t[:, 0:32]
    bw_t = gwbw_t[:, 32:64]
    magic_t = singles.tile([P, 1], U32)
    nc.vector.memset(magic_t[:], MAGIC)

    xsf = xs.rearrange("p d h w -> p (d h w)")
    outf = out_ap.rearrange("p d h w -> p d (h w)")  # d' dim = 8 = NCHUNK*2

    state = {}

    NDMAX = 4
    RMAX = NDMAX * H
    CMAX = RMAX * W
    NQMAX = (NDMAX // 2) * (H // 2)

    def emit_A(k):
        d0, nd = CHUNKS[k]
        ROWS = nd * H
        CH = ROWS * W
        DC = nd
        NQ = (DC // 2) * (H // 2)
        # DMA + ACT square + DVE: gamma/wpool/tree/newton; GP: smalls
        tXf = xpool.tile([P, 2 * CMAX], BF16, tag="tX")
        tx0 = tXf[:, 0:CH]
        tx1 = tXf[:, CH : 2 * CH]
        nc.sync.dma_start(out=tx0, in_=xsf[:, d0 * H * W : (d0 + nd) * H * W])
        nc.scalar.activation(tx1, tx0, AF.Square)

        # g = x * gamma_rep ; s0 = g_lo + g_hi   (DVE bf16 2x)
        gf = gpool.tile([P, RMAX, W], BF16, tag="g")
        g = gf[:, 0:ROWS, :]
        nc.vector.tensor_tensor(
            out=g,
            in0=tx0.rearrange("p (r w) -> p r w", w=W),
            in1=grep_t[:].unsqueeze(1).to_broadcast((P, ROWS, W)),
            op=OP.mult,
        )
        s0f = spool.tile([P, RMAX, 32], BF16, tag="s0")
        s0 = s0f[:, 0:ROWS, :]
        nc.vector.tensor_tensor(
            out=s0, in0=g[:, :, 0:32], in1=g[:, :, 32:64], op=OP.add,
        )

        # stats tree: 6 levels of pairwise adds over [x | x^2]
        t_in = tXf[:, 0 : 2 * CH].rearrange("p (q w) -> p q w", w=W)
        for li, wd in enumerate((32, 16, 8, 4, 2, 1)):
            pool_l = t6pool if wd == 1 else tpool
            tlf = pool_l.tile([P, 2 * RMAX, wd], F32 if wd == 1 else BF16,
                              tag=f"tree{li}")
            tl = tlf[:, 0 : 2 * ROWS, :]
            nc.vector.tensor_tensor(
                out=tl, in0=t_in[:, :, 0:wd], in1=t_in[:, :, wd : 2 * wd],
                op=OP.add,
            )
            t_in = tl
        r1 = t_in[:, 0:ROWS, 0]           # [P, ROWS] f32: sum x
        r2 = t_in[:, ROWS : 2 * ROWS, 0]  # [P, ROWS] f32: sum x^2

        def sm(tag, dt=F32):
            return small.tile([P, RMAX], dt, tag=tag, name=tag)[:, 0:ROWS]

        # rsqrt(64*var) via bit magic + 1 Newton step (DVE smalls, f32)
        msq = sm("msq")
        nc.vector.tensor_tensor(out=msq, in0=r1, in1=r1, op=OP.mult)
        wv2 = sm("wv2")
        nc.vector.scalar_tensor_tensor(
            out=wv2, in0=msq, scalar=-1.0 / W, in1=r2,
            op0=OP.mult, op1=OP.add,
        )
        yi = sm("yi", U32)
        nc.vector.tensor_scalar(
            out=yi, in0=wv2.bitcast(U32), scalar1=1, scalar2=None,
            op0=OP.logical_shift_right,
        )
        y0 = sm("y0", U32)
        nc.vector.tensor_tensor(
            out=y0, in0=magic_t[:].to_broadcast((P, ROWS)), in1=yi,
            op=OP.subtract,
        )
        ys = y0.bitcast(F32)
        a = sm("nta")
        nc.vector.tensor_tensor(out=a, in0=ys, in1=ys, op=OP.mult)
        b = sm("ntb")
        nc.vector.scalar_tensor_tensor(
            out=b, in0=a, scalar=-0.5, in1=wv2, op0=OP.mult, op1=OP.mult
        )
        y = sm("nty")
        nc.vector.scalar_tensor_tensor(
            out=y, in0=b, scalar=1.5, in1=ys, op0=OP.add, op1=OP.mult
        )

        # sr = s0 * y. ACT (which has slack) materializes the y broadcast
        # into packed bf16 so the multiply runs as a 2x TT; the last chunk
        # takes the direct 1x path to keep the drain chain short.
        srf = spool.tile([P, RMAX, 32], BF16, tag="sr")
        sr = srf[:, 0:ROWS, :]
        if k < len(CHUNKS) - 1:
            yrep = gpool.tile([P, RMAX, W], BF16, tag="g", name="yrep")
            nc.scalar.activation(
                yrep[:, 0:ROWS, 0:32],
                y.unsqueeze(2).to_broadcast((P, ROWS, 32)),
                AF.Copy,
            )
            nc.vector.tensor_tensor(
                out=sr, in0=s0, in1=yrep[:, 0:ROWS, 0:32], op=OP.mult,
            )
        else:
            nc.vector.tensor_tensor(
                out=sr, in0=s0,
                in1=y.unsqueeze(2).to_broadcast((P, ROWS, 32)), op=OP.mult,
            )
        sr4 = sr.rearrange("p (s h) w -> p s (h w)", s=DC)
        xdf = spool.tile([P, NDMAX // 2, H * 32], BF16, tag="xd")
        xd = xdf[:, 0 : DC // 2, :]
        nc.vector.tensor_tensor(
            out=xd, in0=sr4[:, 0 : DC // 2, :], in1=sr4[:, DC // 2 : DC, :],
            op=OP.add,
        )
        xhf = spool.tile([P, NDMAX // 2, HW2], BF16, tag="xh")
        xh = xhf[:, 0 : DC // 2, :]
        nc.vector.tensor_tensor(
            out=xh, in0=xd[:, :, 0:HW2], in1=xd[:, :, HW2 : 2 * HW2],
            op=OP.add,
        )
        # GPSIMD small burst: mrs/m1/mq/corr
        mrs = sm("mrs")
        nc.vector.tensor_tensor(out=mrs, in0=r1, in1=y, op=OP.mult)
        mrs4 = mrs.rearrange("p (s h) -> p s h", s=DC)
        m1 = small.tile([P, NDMAX // 2, H], F32, tag="m1", name="m1")[:, 0 : DC // 2, :]
        nc.vector.tensor_tensor(
            out=m1, in0=mrs4[:, 0 : DC // 2, :], in1=mrs4[:, DC // 2 : DC, :],
            op=OP.add,
        )
        mq = small.tile([P, NDMAX // 2, H // 2], F32, tag="mq", name="mq")[:, 0 : DC // 2, :]
        nc.vector.tensor_tensor(
            out=mq, in0=m1[:, :, 0 : H // 2], in1=m1[:, :, H // 2 : H],
            op=OP.add,
        )
        corr = spool.tile([P, NQMAX, 32], BF16, tag="corr", name="corr")[:, 0:NQ, :]
        mq_b = mq.rearrange("p s h -> p (s h)").unsqueeze(2).to_broadcast((P, NQ, 32))
        if k < len(CHUNKS) - 1:
            mqrep = opool.tile([P, NQMAX * 32], BF16, tag="pre", name="mqrep")
            mqr = mqrep[:, 0 : NQ * 32].rearrange("p (a b) -> p a b", b=32)
            nc.scalar.activation(mqr, mq_b, AF.Copy)
            nc.vector.tensor_tensor(
                out=corr, in0=gw_t.unsqueeze(1).to_broadcast((P, NQ, 32)),
                in1=mqr, op=OP.mult,
            )
        else:
            nc.vector.tensor_tensor(
                out=corr, in0=gw_t.unsqueeze(1).to_broadcast((P, NQ, 32)),
                in1=mq_b, op=OP.mult,
            )
        state[k] = (xh, corr, d0, nd)

    def emit_B(k):
        xh, corr, d0, nd = state.pop(k)
        NQ = (nd // 2) * (H // 2)
        pre = opool.tile([P, NQMAX * 32], BF16, tag="pre", name="pre")[:, 0 : NQ * 32]
        nc.vector.tensor_tensor(
            out=pre,
            in0=xh.rearrange("p a b -> p (a b)"),
            in1=corr.rearrange("p a b -> p (a b)"),
            op=OP.subtract,
        )
        pre2 = opool.tile([P, NQMAX, 32], BF16, tag="pre2", name="pre2")[:, 0:NQ, :]
        nc.vector.tensor_tensor(
            out=pre2,
            in0=pre.rearrange("p (a b) -> p a b", b=32),
            in1=bw_t.unsqueeze(1).to_broadcast((P, NQ, 32)),
            op=OP.add,
        )
        res = opool.tile([P, NQMAX * 32], BF16, tag="res", name="res")[:, 0 : NQ * 32]
        nc.scalar.activation(
            res, pre2.rearrange("p a b -> p (a b)"), AF.Gelu
        )
        nc.sync.dma_start(
            out=outf[:, d0 // 2 : d0 // 2 + nd // 2, :],
            in_=res.rearrange("p (a b) -> p a b", b=HW2),
        )

    # software pipeline: A(0) A(1) B(0) A(2) B(1) ...
    NC_ = len(CHUNKS)
    emit_A(0)
    emit_A(1)
    for k in range(NC_):
        if k + 2 < NC_:
            emit_A(k + 2)
        emit_B(k)


_CACHE: dict = {}


def _get_compiled():
    if "nc" not in _CACHE:
        nc = bacc.Bacc("TRN2", target_bir_lowering=False, debug=False)
        xs = nc.dram_tensor("xs", [P, D, H, W], BF16, kind="ExternalInput").ap()
        cons = nc.dram_tensor("cons", [2, 64], BF16, kind="ExternalInput").ap()
        out = nc.dram_tensor(
            "out", [P, D // 2, H // 2, W // 2], BF16, kind="ExternalOutput"
        ).ap()
        from contextlib import ExitStack

        with tile.TileContext(nc) as tc, ExitStack() as ctx:
            _kernel_body(ctx, tc, out, xs, cons)
        nc.compile()
        _CACHE["nc"] = nc
    return _CACHE["nc"]


# host-side index permutations: even|odd halves for d (per chunk), h, w
_DORD = np.concatenate([
    np.concatenate([np.arange(d0, d0 + nd, 2), np.arange(d0 + 1, d0 + nd, 2)])
    for d0, nd in CHUNKS
])
_HORD = np.concatenate([np.arange(0, H, 2), np.arange(1, H, 2)])
_WORD = np.concatenate([np.arange(0, W, 2), np.arange(1, W, 2)])


def _make_cons(gamma: np.ndarray, beta: np.ndarray) -> np.ndarray:
    ga = gamma[0::2].astype(np.float64)
    go = gamma[1::2].astype(np.float64)
    grep = np.concatenate([ga, go])                      # raw, deinterleaved
    gw = (ga + go) / float(W)                            # gw'' = (ga+go)/64
    bw = 0.5 * (beta[0::2] + beta[1::2]).astype(np.float64)
    row1 = np.concatenate([gw, bw])
    return np.stack([grep, row1]).astype(NP_BF16)


def kernel(x, sum_weight, gamma, beta, trace=False):
    del sum_weight  # cancels exactly in LayerNorm (shift invariance)
    nc = _get_compiled()
    x = np.asarray(x)
    # permute d/h/w into even|odd halves, cast bf16
    xp = x[:, :, _DORD][:, :, :, _HORD][:, :, :, :, _WORD].astype(NP_BF16)
    cons = _make_cons(np.asarray(gamma), np.asarray(beta))
    in_maps = []
    for core in range(NCORES):
        shard = np.ascontiguousarray(
            xp[core * NPER : (core + 1) * NPER].reshape(P, D, H, W)
        )
        in_maps.append({"xs": shard, "cons": cons})
    res = run_bass_kernel_spmd(nc, in_maps, core_ids=list(range(NCORES)), trace=trace)
    out = np.concatenate(
        [
            res.results[i]["out"]
            .reshape(NPER, C, D // 2, H // 2, W // 2)
            .astype(np.float32)
            for i in range(NCORES)
        ],
        axis=0,
    )
    if trace:
        return out, res
    return out


if __name__ == "__main__":
    rng = np.random.default_rng(0)
    x = rng.standard_normal((N, C, D, H, W), dtype=np.float32)
    sw = rng.standard_normal((1,)).astype(np.float32)
    gamma = rng.random((W,), dtype=np.float32)
    beta = rng.standard_normal((W,)).astype(np.float32)
    y = kernel(x, sw, gamma, beta)
    print(y.shape, y.dtype)


# revision 28
# speedup vs baseline: 1.0611x; 1.0382x over previous
"""Trainium2 Bass kernel: x + s -> LayerNorm(W) -> 2x2x2 avgpool -> exact GELU.

Input  x: (32, 32, 16, 32, 64) f32, sum_weight (1,), gamma (64,), beta (64,)
Output:   (32, 32, 8, 16, 32) f32

Math:
  LN is shift-invariant, so sum_weight cancels exactly.
  pooled[q, w'] = sum_{r in quad} y_r (ga x_e + go x_o)[w'] - gw''[w'] mq[q] + bw[w']
    y_r   = rho_r / 8 = rsqrt(64 var_r)   (rsqrt via bit-magic + 1 Newton step)
    mq[q] = sum_{r in quad} r1_r y_r,  gw'' = (ga+go)/64,  bw = (be+bo)/2
  out = Gelu(pooled)

Layout: data-parallel over batch N (4 per core x 8 cores). Partitions = the
128 (n, c) pairs. Host pre-permutes d/h/w into even|odd halves and converts
to bf16, so every bulk op is a contiguous-half TENSOR_TENSOR that hits the
DVE 2x bf16 fast path (0.54 ns/elem measured on HW). Per-row stats come
from a 6-level pairwise add tree over a flat tile holding [x | x^2]; Square
and Gelu share one ACT table (one table load total). Everything else runs
on DVE: GPSIMD TT is ~2.6 ns/elem AND stalls concurrent DVE ops via shared
SBUF ports, so offloading there is net-negative. Chunks are variable-size
(small first/last) to cut pipeline fill and drain; issue order interleaves
A-phase of chunk k+2 with B-phase of chunk k.
"""

import numpy as np

import concourse.bacc as bacc
import concourse.bass as bass
import concourse.tile as tile
from concourse import mybir
from concourse.bass_utils import run_bass_kernel_spmd

P = 128
N, C, D, H, W = 32, 32, 16, 32, 64
NCORES = 8
NPER = N // NCORES
EPS = 1e-5
F32 = mybir.dt.float32
BF16 = mybir.dt.bfloat16
U32 = mybir.dt.uint32
NP_BF16 = mybir.dt.np(BF16)

# variable-size chunks (in d-planes): small first/last to cut pipeline
# fill and drain; middles large for low instruction overhead
CHUNKS = [(0, 2), (2, 4), (6, 4), (10, 4), (14, 2)]
HW2 = (H // 2) * 32   # 512: h' x w' per d' slot
MAGIC = 0x5F3759DF

AF = mybir.ActivationFunctionType
OP = mybir.AluOpType


def _kernel_body(ctx, tc: tile.TileContext, out_ap: bass.AP, xs: bass.AP, cons: bass.AP):
    nc = tc.nc

    singles = ctx.enter_context(tc.tile_pool(name="singles", bufs=1))
    xpool = ctx.enter_context(tc.tile_pool(name="xpool", bufs=2))
    tpool = ctx.enter_context(tc.tile_pool(name="tpool", bufs=1))
    t6pool = ctx.enter_context(tc.tile_pool(name="t6pool", bufs=3))
    gpool = ctx.enter_context(tc.tile_pool(name="gpool", bufs=2))
    spool = ctx.enter_context(tc.tile_pool(name="spool", bufs=2))
    small = ctx.enter_context(tc.tile_pool(name="small", bufs=2))
    opool = ctx.enter_context(tc.tile_pool(name="opool", bufs=2))

    xsf = xs.rearrange("p d h w -> p (d h w)")
    outf = out_ap.rearrange("p d h w -> p d (h w)")  # d' dim = 8 = NCHUNK*2

    state = {}
    prefetched = {}

    NDMAX = 4
    RMAX = NDMAX * H
    CMAX = RMAX * W
    NQMAX = (NDMAX // 2) * (H // 2)

    # issue chunk-0's data DMA before anything else (startup critical path)
    d0_0, nd_0 = CHUNKS[0]
    tX0 = xpool.tile([P, 2 * CMAX], BF16, tag="tX")
    nc.sync.dma_start(
        out=tX0[:, 0 : nd_0 * H * W],
        in_=xsf[:, d0_0 * H * W : (d0_0 + nd_0) * H * W],
    )
    prefetched[0] = tX0

    # constants (bf16), broadcast to all partitions
    grep_t = singles.tile([P, 64], BF16)   # [ga(32) | go(32)], raw gamma deint
    nc.sync.dma_start(out=grep_t[:], in_=cons[0:1, :].to_broadcast((P, 64)))
    gwbw_t = singles.tile([P, 64], BF16)   # [gw''(32) | bw(32)]
    nc.sync.dma_start(out=gwbw_t[:], in_=cons[1:2, :].to_broadcast((P, 64)))
    gw_t = gwbw_t[:, 0:32]
    bw_t = gwbw_t[:, 32:64]
    magic_t = singles.tile([P, 1], U32)
    nc.vector.memset(magic_t[:], MAGIC)

    def emit_A(k):
        d0, nd = CHUNKS[k]
        ROWS = nd * H
        CH = ROWS * W
        DC = nd
        NQ = (DC // 2) * (H // 2)
        # DMA + ACT square + DVE: gamma/wpool/tree/newton
        if k in prefetched:
            tXf = prefetched.pop(k)
        else:
            tXf = xpool.tile([P, 2 * CMAX], BF16, tag="tX")
            nc.sync.dma_start(
                out=tXf[:, 0:CH], in_=xsf[:, d0 * H * W : (d0 + nd) * H * W]
            )
        tx0 = tXf[:, 0:CH]
        tx1 = tXf[:, CH : 2 * CH]
        nc.scalar.activation(tx1, tx0, AF.Square)

        # g = x * gamma_rep ; s0 = g_lo + g_hi   (DVE bf16 2x)
        gf = gpool.tile([P, RMAX, W], BF16, tag="g")
        g = gf[:, 0:ROWS, :]
        nc.vector.tensor_tensor(
            out=g,
            in0=tx0.rearrange("p (r w) -> p r w", w=W),
            in1=grep_t[:].unsqueeze(1).to_broadcast((P, ROWS, W)),
            op=OP.mult,
        )
        s0f = spool.tile([P, RMAX, 32], BF16, tag="s0")
        s0 = s0f[:, 0:ROWS, :]
        nc.vector.tensor_tensor(
            out=s0, in0=g[:, :, 0:32], in1=g[:, :, 32:64], op=OP.add,
        )

        # stats tree: 6 levels of pairwise adds over [x | x^2]
        t_in = tXf[:, 0 : 2 * CH].rearrange("p (q w) -> p q w", w=W)
        for li, wd in enumerate((32, 16, 8, 4, 2, 1)):
            pool_l = t6pool if wd == 1 else tpool
            tlf = pool_l.tile([P, 2 * RMAX, wd], F32 if wd == 1 else BF16,
                              tag=f"tree{li}")
            tl = tlf[:, 0 : 2 * ROWS, :]
            nc.vector.tensor_tensor(
                out=tl, in0=t_in[:, :, 0:wd], in1=t_in[:, :, wd : 2 * wd],
                op=OP.add,
            )
            t_in = tl
        r1 = t_in[:, 0:ROWS, 0]           # [P, ROWS] f32: sum x
        r2 = t_in[:, ROWS : 2 * ROWS, 0]  # [P, ROWS] f32: sum x^2

        def sm(tag, dt=F32):
            return small.tile([P, RMAX], dt, tag=tag, name=tag)[:, 0:ROWS]

        # rsqrt(64*var) via bit magic + 1 Newton step (DVE smalls, f32)
        msq = sm("msq")
        nc.vector.tensor_tensor(out=msq, in0=r1, in1=r1, op=OP.mult)
        wv2 = sm("wv2")
        nc.vector.scalar_tensor_tensor(
            out=wv2, in0=msq, scalar=-1.0 / W, in1=r2,
            op0=OP.mult, op1=OP.add,
        )
        yi = sm("yi", U32)
        nc.vector.tensor_scalar(
            out=yi, in0=wv2.bitcast(U32), scalar1=1, scalar2=None,
            op0=OP.logical_shift_right,
        )
        y0 = sm("y0", U32)
        nc.vector.tensor_tensor(
            out=y0, in0=magic_t[:].to_broadcast((P, ROWS)), in1=yi,
            op=OP.subtract,
        )
        ys = y0.bitcast(F32)
        a = sm("nta")
        nc.vector.tensor_tensor(out=a, in0=ys, in1=ys, op=OP.mult)
        b = sm("ntb")
        nc.vector.scalar_tensor_tensor(
            out=b, in0=a, scalar=-0.5, in1=wv2, op0=OP.mult, op1=OP.mult
        )
        y = sm("nty")
        nc.vector.scalar_tensor_tensor(
            out=y, in0=b, scalar=1.5, in1=ys, op0=OP.add, op1=OP.mult
        )

        # sr = s0 * y. ACT (which has slack) materializes the y broadcast
        # into packed bf16 so the multiply runs as a 2x TT; the last chunk
        # takes the direct 1x path to keep the drain chain short.
        srf = spool.tile([P, RMAX, 32], BF16, tag="sr")
        sr = srf[:, 0:ROWS, :]
        if k < len(CHUNKS) - 1:
            yrep = gpool.tile([P, RMAX, W], BF16, tag="g", name="yrep")
            nc.scalar.activation(
                yrep[:, 0:ROWS, 0:32],
                y.unsqueeze(2).to_broadcast((P, ROWS, 32)),
                AF.Copy,
            )
            nc.vector.tensor_tensor(
                out=sr, in0=s0, in1=yrep[:, 0:ROWS, 0:32], op=OP.mult,
            )
        else:
            nc.vector.tensor_tensor(
                out=sr, in0=s0,
                in1=y.unsqueeze(2).to_broadcast((P, ROWS, 32)), op=OP.mult,
            )
        sr4 = sr.rearrange("p (s h) w -> p s (h w)", s=DC)
        xdf = spool.tile([P, NDMAX // 2, H * 32], BF16, tag="xd")
        xd = xdf[:, 0 : DC // 2, :]
        nc.vector.tensor_tensor(
            out=xd, in0=sr4[:, 0 : DC // 2, :], in1=sr4[:, DC // 2 : DC, :],
            op=OP.add,
        )
        xhf = spool.tile([P, NDMAX // 2, HW2], BF16, tag="xh")
        xh = xhf[:, 0 : DC // 2, :]
        nc.vector.tensor_tensor(
            out=xh, in0=xd[:, :, 0:HW2], in1=xd[:, :, HW2 : 2 * HW2],
            op=OP.add,
        )
        # GPSIMD small burst: mrs/m1/mq/corr
        mrs = sm("mrs")
        nc.vector.tensor_tensor(out=mrs, in0=r1, in1=y, op=OP.mult)
        mrs4 = mrs.rearrange("p (s h) -> p s h", s=DC)
        m1 = small.tile([P, NDMAX // 2, H], F32, tag="m1", name="m1")[:, 0 : DC // 2, :]
        nc.vector.tensor_tensor(
            out=m1, in0=mrs4[:, 0 : DC // 2, :], in1=mrs4[:, DC // 2 : DC, :],
            op=OP.add,
        )
        mq = small.tile([P, NDMAX // 2, H // 2], F32, tag="mq", name="mq")[:, 0 : DC // 2, :]
        nc.vector.tensor_tensor(
            out=mq, in0=m1[:, :, 0 : H // 2], in1=m1[:, :, H // 2 : H],
            op=OP.add,
        )
        corr = spool.tile([P, NQMAX, 32], BF16, tag="corr", name="corr")[:, 0:NQ, :]
        mq_b = mq.rearrange("p s h -> p (s h)").unsqueeze(2).to_broadcast((P, NQ, 32))
        if k < len(CHUNKS) - 1:
            mqrep = opool.tile([P, NQMAX * 32], BF16, tag="pre", name="mqrep")
            mqr = mqrep[:, 0 : NQ * 32].rearrange("p (a b) -> p a b", b=32)
            nc.scalar.activation(mqr, mq_b, AF.Copy)
            nc.vector.tensor_tensor(
                out=corr, in0=gw_t.unsqueeze(1).to_broadcast((P, NQ, 32)),
                in1=mqr, op=OP.mult,
            )
        else:
            nc.vector.tensor_tensor(
                out=corr, in0=gw_t.unsqueeze(1).to_broadcast((P, NQ, 32)),
                in1=mq_b, op=OP.mult,
            )
        state[k] = (xh, corr, d0, nd)

    def emit_B(k):
        xh, corr, d0, nd = state.pop(k)
        NQ = (nd // 2) * (H // 2)
        pre = opool.tile([P, NQMAX * 32], BF16, tag="pre", name="pre")[:, 0 : NQ * 32]
        nc.vector.tensor_tensor(
            out=pre,
            in0=xh.rearrange("p a b -> p (a b)"),
            in1=corr.rearrange("p a b -> p (a b)"),
            op=OP.subtract,
        )
        pre2 = opool.tile([P, NQMAX, 32], BF16, tag="pre2", name="pre2")[:, 0:NQ, :]
        nc.vector.tensor_tensor(
            out=pre2,
            in0=pre.rearrange("p (a b) -> p a b", b=32),
            in1=bw_t.unsqueeze(1).to_broadcast((P, NQ, 32)),
            op=OP.add,
        )
        res = opool.tile([P, NQMAX * 32], BF16, tag="res", name="res")[:, 0 : NQ * 32]
        nc.scalar.activation(
            res, pre2.rearrange("p a b -> p (a b)"), AF.Gelu
        )
        nc.sync.dma_start(
            out=outf[:, d0 // 2 : d0 // 2 + nd // 2, :],
            in_=res.rearrange("p (a b) -> p a b", b=HW2),
        )

    # software pipeline: A(0) A(1) B(0) A(2) B(1) ...
    NC_ = len(CHUNKS)
    emit_A(0)
    emit_A(1)
    for k in range(NC_):
        if k + 2 < NC_:
            emit_A(k + 2)
        emit_B(k)


_CACHE: dict = {}


def _get_compiled():
    if "nc" not in _CACHE:
        nc = bacc.Bacc("TRN2", target_bir_lowering=False, debug=False)
        xs = nc.dram_tensor("xs", [P, D, H, W], BF16, kind="ExternalInput").ap()
        cons = nc.dram_tensor("cons", [2, 64], BF16, kind="ExternalInput").ap()
        out = nc.dram_tensor(
            "out", [P, D // 2, H // 2, W // 2], BF16, kind="ExternalOutput"
        ).ap()
        from contextlib import ExitStack

        with tile.TileContext(nc) as tc, ExitStack() as ctx:
            _kernel_body(ctx, tc, out, xs, cons)
        nc.compile()
        _CACHE["nc"] = nc
    return _CACHE["nc"]


# host-side index permutations: even|odd halves for d (per chunk), h, w
_DORD = np.concatenate([
    np.concatenate([np.arange(d0, d0 + nd, 2), np.arange(d0 + 1, d0 + nd, 2)])
    for d0, nd in CHUNKS
])
_HORD = np.concatenate([np.arange(0, H, 2), np.arange(1, H, 2)])
_WORD = np.concatenate([np.arange(0, W, 2), np.arange(1, W, 2)])


def _make_cons(gamma: np.ndarray, beta: np.ndarray) -> np.ndarray:
    ga = gamma[0::2].astype(np.float64)
    go = gamma[1::2].astype(np.float64)
    grep = np.concatenate([ga, go])                      # raw, deinterleaved
    gw = (ga + go) / float(W)                            # gw'' = (ga+go)/64
    bw = 0.5 * (beta[0::2] + beta[1::2]).astype(np.float64)
    row1 = np.concatenate([gw, bw])
    return np.stack([grep, row1]).astype(NP_BF16)


def kernel(x, sum_weight, gamma, beta, trace=False):
    del sum_weight  # cancels exactly in LayerNorm (shift invariance)
    nc = _get_compiled()
    x = np.asarray(x)
    # permute d/h/w into even|odd halves, cast bf16
    xp = x[:, :, _DORD][:, :, :, _HORD][:, :, :, :, _WORD].astype(NP_BF16)
    cons = _make_cons(np.asarray(gamma), np.asarray(beta))
    in_maps = []
    for core in range(NCORES):
        shard = np.ascontiguousarray(
            xp[core * NPER : (core + 1) * NPER].reshape(P, D, H, W)
        )
        in_maps.append({"xs": shard, "cons": cons})
    res = run_bass_kernel_spmd(nc, in_maps, core_ids=list(range(NCORES)), trace=trace)
    out = np.concatenate(
        [
            res.results[i]["out"]
            .reshape(NPER, C, D // 2, H // 2, W // 2)
            .astype(np.float32)
            for i in range(NCORES)
        ],
        axis=0,
    )
    if trace:
        return out, res
    return out


if __name__ == "__main__":
    rng = np.random.default_rng(0)
    x = rng.standard_normal((N, C, D, H, W), dtype=np.float32)
    sw = rng.standard_normal((1,)).astype(np.float32)
    gamma = rng.random((W,), dtype=np.float32)
    beta = rng.standard_normal((W,)).astype(np.float32)
    y = kernel(x, sw, gamma, beta)
    print(y.shape, y.dtype)


# revision 30
# speedup vs baseline: 1.0720x; 1.0103x over previous
"""Trainium2 Bass kernel: x + s -> LayerNorm(W) -> 2x2x2 avgpool -> exact GELU.

Input  x: (32, 32, 16, 32, 64) f32, sum_weight (1,), gamma (64,), beta (64,)
Output:   (32, 32, 8, 16, 32) f32

Math:
  LN is shift-invariant, so sum_weight cancels exactly.
  pooled[q, w'] = sum_{r in quad} y_r (ga x_e + go x_o)[w'] - gw''[w'] mq[q] + bw[w']
    y_r   = rho_r / 8 = rsqrt(64 var_r)   (rsqrt via bit-magic + 1 Newton step)
    mq[q] = sum_{r in quad} r1_r y_r,  gw'' = (ga+go)/64,  bw = (be+bo)/2
  out = Gelu(pooled)

Layout: data-parallel over batch N (4 per core x 8 cores). Partitions = the
128 (n, c) pairs. Host pre-permutes d/h/w into even|odd halves and converts
to bf16, so every bulk op is a contiguous-half TENSOR_TENSOR that hits the
DVE 2x bf16 fast path (0.54 ns/elem measured on HW). Per-row stats come
from a 6-level pairwise add tree over a flat tile holding [x | x^2]; Square
and Gelu share one ACT table (one table load total). Everything else runs
on DVE: GPSIMD TT is ~2.6 ns/elem AND stalls concurrent DVE ops via shared
SBUF ports, so offloading there is net-negative. Chunks are variable-size
(small first/last) to cut pipeline fill and drain; issue order interleaves
A-phase of chunk k+2 with B-phase of chunk k.
"""

import numpy as np

import concourse.bacc as bacc
import concourse.bass as bass
import concourse.tile as tile
from concourse import mybir
from concourse.bass_utils import run_bass_kernel_spmd

P = 128
N, C, D, H, W = 32, 32, 16, 32, 64
NCORES = 8
NPER = N // NCORES
EPS = 1e-5
F32 = mybir.dt.float32
BF16 = mybir.dt.bfloat16
U32 = mybir.dt.uint32
NP_BF16 = mybir.dt.np(BF16)

# variable-size chunks (in d-planes): small first/last to cut pipeline
# fill and drain; middles large for low instruction overhead
CHUNKS = [(0, 2), (2, 4), (6, 4), (10, 4), (14, 2)]
HW2 = (H // 2) * 32   # 512: h' x w' per d' slot
MAGIC = 0x5F3759DF

AF = mybir.ActivationFunctionType
OP = mybir.AluOpType


def _kernel_body(ctx, tc: tile.TileContext, out_ap: bass.AP, xs: bass.AP, cons: bass.AP):
    nc = tc.nc

    singles = ctx.enter_context(tc.tile_pool(name="singles", bufs=1))
    xpool = ctx.enter_context(tc.tile_pool(name="xpool", bufs=2))
    tpool = ctx.enter_context(tc.tile_pool(name="tpool", bufs=1))
    t6pool = ctx.enter_context(tc.tile_pool(name="t6pool", bufs=2))
    gpool = ctx.enter_context(tc.tile_pool(name="gpool", bufs=2))
    spool = ctx.enter_context(tc.tile_pool(name="spool", bufs=2))
    small = ctx.enter_context(tc.tile_pool(name="small", bufs=1))
    opool = ctx.enter_context(tc.tile_pool(name="opool", bufs=2))

    xsf = xs.rearrange("p d h w -> p (d h w)")
    outf = out_ap.rearrange("p d h w -> p d (h w)")  # d' dim = 8 = NCHUNK*2

    state = {}
    stateB = {}
    prefetched = {}

    NDMAX = 4
    RMAX = NDMAX * H
    CMAX = RMAX * W
    NQMAX = (NDMAX // 2) * (H // 2)

    # issue chunk-0's data DMA before anything else (startup critical path)
    d0_0, nd_0 = CHUNKS[0]
    tX0 = xpool.tile([P, 2 * CMAX], BF16, tag="tX")
    nc.sync.dma_start(
        out=tX0[:, 0 : nd_0 * H * W],
        in_=xsf[:, d0_0 * H * W : (d0_0 + nd_0) * H * W],
    )
    prefetched[0] = tX0

    # constants (bf16), broadcast to all partitions
    grep_t = singles.tile([P, 64], BF16)   # [ga(32) | go(32)], raw gamma deint
    nc.sync.dma_start(out=grep_t[:], in_=cons[0:1, :].to_broadcast((P, 64)))
    gwbw_t = singles.tile([P, 64], BF16)   # [gw''(32) | bw(32)]
    nc.sync.dma_start(out=gwbw_t[:], in_=cons[1:2, :].to_broadcast((P, 64)))
    gw_t = gwbw_t[:, 0:32]
    bw_t = gwbw_t[:, 32:64]
    magic_t = singles.tile([P, 1], U32)
    nc.vector.memset(magic_t[:], MAGIC)

    def emit_A(k, t6p, off):
        d0, nd = CHUNKS[k]
        ROWS = nd * H
        CH = ROWS * W
        DC = nd
        NQ = (DC // 2) * (H // 2)
        # DMA + ACT square + DVE: gamma/wpool/tree/newton
        if k in prefetched:
            tXf = prefetched.pop(k)
        else:
            tXf = xpool.tile([P, 2 * CMAX], BF16, tag="tX")
            nc.sync.dma_start(
                out=tXf[:, 0:CH], in_=xsf[:, d0 * H * W : (d0 + nd) * H * W]
            )
        tx0 = tXf[:, 0:CH]
        tx1 = tXf[:, CH : 2 * CH]
        nc.scalar.activation(tx1, tx0, AF.Square)

        # g = x * gamma_rep ; s0 = g_lo + g_hi   (DVE bf16 2x)
        gf = gpool.tile([P, RMAX, W], BF16, tag="g")
        g = gf[:, 0:ROWS, :]
        nc.vector.tensor_tensor(
            out=g,
            in0=tx0.rearrange("p (r w) -> p r w", w=W),
            in1=grep_t[:].unsqueeze(1).to_broadcast((P, ROWS, W)),
            op=OP.mult,
        )
        s0f = spool.tile([P, RMAX, 32], BF16, tag="s0")
        s0 = s0f[:, 0:ROWS, :]
        nc.vector.tensor_tensor(
            out=s0, in0=g[:, :, 0:32], in1=g[:, :, 32:64], op=OP.add,
        )

        # stats tree: 5 levels of pairwise adds over [x | x^2], then L6
        # writes f32 sums into this pair's shared stats tile at `off`
        t_in = tXf[:, 0 : 2 * CH].rearrange("p (q w) -> p q w", w=W)
        for li, wd in enumerate((32, 16, 8, 4, 2)):
            tlf = tpool.tile([P, 2 * RMAX, wd], BF16, tag=f"tree{li}")
            tl = tlf[:, 0 : 2 * ROWS, :]
            nc.vector.tensor_tensor(
                out=tl, in0=t_in[:, :, 0:wd], in1=t_in[:, :, wd : 2 * wd],
                op=OP.add,
            )
            t_in = tl
        l5 = t_in.rearrange("p (t r) w -> p t r w", t=2)
        nc.vector.tensor_tensor(
            out=t6p[:, :, off : off + ROWS],
            in0=l5[:, :, :, 0],
            in1=l5[:, :, :, 1],
            op=OP.add,
        )

        state[k] = (s0, t6p, off, d0, nd)


    def emit_N(t6p, tot):
        # batched rsqrt(64*var) via bit magic + 1 Newton step over `tot` rows
        def sm(tag, dt=F32):
            return small.tile([P, 2 * RMAX], dt, tag=tag, name=tag)[:, 0:tot]

        r1 = t6p[:, 0, 0:tot]
        r2 = t6p[:, 1, 0:tot]
        msq = sm("msq")
        nc.vector.tensor_tensor(out=msq, in0=r1, in1=r1, op=OP.mult)
        wv2 = sm("wv2")
        nc.vector.scalar_tensor_tensor(
            out=wv2, in0=msq, scalar=-1.0 / W, in1=r2,
            op0=OP.mult, op1=OP.add,
        )
        yi = sm("yi", U32)
        nc.vector.tensor_scalar(
            out=yi, in0=wv2.bitcast(U32), scalar1=1, scalar2=None,
            op0=OP.logical_shift_right,
        )
        y0 = sm("y0", U32)
        nc.vector.tensor_tensor(
            out=y0, in0=magic_t[:].to_broadcast((P, tot)), in1=yi,
            op=OP.subtract,
        )
        ys = y0.bitcast(F32)
        a = sm("nta")
        nc.vector.tensor_tensor(out=a, in0=ys, in1=ys, op=OP.mult)
        b = sm("ntb")
        nc.vector.scalar_tensor_tensor(
            out=b, in0=a, scalar=-0.5, in1=wv2, op0=OP.mult, op1=OP.mult
        )
        yt = sm("nty")
        nc.vector.scalar_tensor_tensor(
            out=yt, in0=b, scalar=1.5, in1=ys, op0=OP.add, op1=OP.mult
        )
        return yt

    def emit_M(k, y_all):
        s0, t6p, off, d0, nd = state.pop(k)
        ROWS = nd * H
        DC = nd
        NQ = (DC // 2) * (H // 2)
        y = y_all[:, off : off + ROWS]
        r1 = t6p[:, 0, off : off + ROWS]
        srf = spool.tile([P, RMAX, 32], BF16, tag="sr")
        sr = srf[:, 0:ROWS, :]
        if k < len(CHUNKS) - 1:
            yrep = gpool.tile([P, RMAX, W], BF16, tag="g", name="yrep")
            nc.scalar.activation(
                yrep[:, 0:ROWS, 0:32],
                y.unsqueeze(2).to_broadcast((P, ROWS, 32)),
                AF.Copy,
            )
            nc.vector.tensor_tensor(
                out=sr, in0=s0, in1=yrep[:, 0:ROWS, 0:32], op=OP.mult,
            )
        else:
            nc.vector.tensor_tensor(
                out=sr, in0=s0,
                in1=y.unsqueeze(2).to_broadcast((P, ROWS, 32)), op=OP.mult,
            )
        sr4 = sr.rearrange("p (s h) w -> p s (h w)", s=DC)
        xdf = spool.tile([P, NDMAX // 2, H * 32], BF16, tag="xd")
        xd = xdf[:, 0 : DC // 2, :]
        nc.vector.tensor_tensor(
            out=xd, in0=sr4[:, 0 : DC // 2, :], in1=sr4[:, DC // 2 : DC, :],
            op=OP.add,
        )
        xhf = spool.tile([P, NDMAX // 2, HW2], BF16, tag="xh")
        xh = xhf[:, 0 : DC // 2, :]
        nc.vector.tensor_tensor(
            out=xh, in0=xd[:, :, 0:HW2], in1=xd[:, :, HW2 : 2 * HW2],
            op=OP.add,
        )
        mrs = small.tile([P, RMAX], F32, tag="mrs", name="mrs")[:, 0:ROWS]
        nc.vector.tensor_tensor(out=mrs, in0=r1, in1=y, op=OP.mult)
        mrs4 = mrs.rearrange("p (s h) -> p s h", s=DC)
        m1 = small.tile([P, NDMAX // 2, H], F32, tag="m1", name="m1")[:, 0 : DC // 2, :]
        nc.vector.tensor_tensor(
            out=m1, in0=mrs4[:, 0 : DC // 2, :], in1=mrs4[:, DC // 2 : DC, :],
            op=OP.add,
        )
        mq = small.tile([P, NDMAX // 2, H // 2], F32, tag="mq", name="mq")[:, 0 : DC // 2, :]
        nc.vector.tensor_tensor(
            out=mq, in0=m1[:, :, 0 : H // 2], in1=m1[:, :, H // 2 : H],
            op=OP.add,
        )
        mq_b = mq.rearrange("p s h -> p (s h)").unsqueeze(2).to_broadcast((P, NQ, 32))
        corr = spool.tile([P, NQMAX, 32], BF16, tag="corr", name="corr")[:, 0:NQ, :]
        if k < len(CHUNKS) - 1:
            mqrep = opool.tile([P, NQMAX * 32], BF16, tag="pre", name="mqrep")
            mqr = mqrep[:, 0 : NQ * 32].rearrange("p (a b) -> p a b", b=32)
            nc.scalar.activation(mqr, mq_b, AF.Copy)
            nc.vector.tensor_tensor(
                out=corr, in0=gw_t.unsqueeze(1).to_broadcast((P, NQ, 32)),
                in1=mqr, op=OP.mult,
            )
        else:
            nc.vector.tensor_tensor(
                out=corr, in0=gw_t.unsqueeze(1).to_broadcast((P, NQ, 32)),
                in1=mq_b, op=OP.mult,
            )
        stateB[k] = (xh, corr, d0, nd)

    def emit_B(k):
        xh, corr, d0, nd = stateB.pop(k)
        NQ = (nd // 2) * (H // 2)
        pre = opool.tile([P, NQMAX * 32], BF16, tag="pre", name="pre")[:, 0 : NQ * 32]
        nc.vector.tensor_tensor(
            out=pre,
            in0=xh.rearrange("p a b -> p (a b)"),
            in1=corr.rearrange("p a b -> p (a b)"),
            op=OP.subtract,
        )
        pre2 = opool.tile([P, NQMAX, 32], BF16, tag="pre2", name="pre2")[:, 0:NQ, :]
        nc.vector.tensor_tensor(
            out=pre2,
            in0=pre.rearrange("p (a b) -> p a b", b=32),
            in1=bw_t.unsqueeze(1).to_broadcast((P, NQ, 32)),
            op=OP.add,
        )
        res = opool.tile([P, NQMAX * 32], BF16, tag="res", name="res")[:, 0 : NQ * 32]
        nc.scalar.activation(
            res, pre2.rearrange("p a b -> p (a b)"), AF.Gelu
        )
        nc.sync.dma_start(
            out=outf[:, d0 // 2 : d0 // 2 + nd // 2, :],
            in_=res.rearrange("p (a b) -> p a b", b=HW2),
        )

    # pairs share one stats tile and one batched newton pass
    # pipeline: A0 A1 N01 M0 A2 M1 B0 A3 N23 M2 B1 A4 M3 B2 N4 M4 B3 B4
    def pair_tile():
        t6p = t6pool.tile([P, 2, 2 * RMAX], F32, tag="t6p", name="t6p")
        return t6p

    r0 = CHUNKS[0][1] * H
    r1_ = CHUNKS[1][1] * H
    r2_ = CHUNKS[2][1] * H
    r3_ = CHUNKS[3][1] * H
    r4_ = CHUNKS[4][1] * H
    tp01 = pair_tile()
    emit_A(0, tp01, 0)
    emit_A(1, tp01, r0)
    y01 = emit_N(tp01, r0 + r1_)
    emit_M(0, y01)
    tp23 = pair_tile()
    emit_A(2, tp23, 0)
    emit_M(1, y01)
    emit_B(0)
    emit_A(3, tp23, r2_)
    y23 = emit_N(tp23, r2_ + r3_)
    emit_M(2, y23)
    emit_B(1)
    tp4 = pair_tile()
    emit_A(4, tp4, 0)
    emit_M(3, y23)
    emit_B(2)
    y4 = emit_N(tp4, r4_)
    emit_M(4, y4)
    emit_B(3)
    emit_B(4)


_CACHE: dict = {}


def _get_compiled():
    if "nc" not in _CACHE:
        nc = bacc.Bacc("TRN2", target_bir_lowering=False, debug=False)
        xs = nc.dram_tensor("xs", [P, D, H, W], BF16, kind="ExternalInput").ap()
        cons = nc.dram_tensor("cons", [2, 64], BF16, kind="ExternalInput").ap()
        out = nc.dram_tensor(
            "out", [P, D // 2, H // 2, W // 2], BF16, kind="ExternalOutput"
        ).ap()
        from contextlib import ExitStack

        with tile.TileContext(nc) as tc, ExitStack() as ctx:
            _kernel_body(ctx, tc, out, xs, cons)
        nc.compile()
        _CACHE["nc"] = nc
    return _CACHE["nc"]


# host-side index permutations: even|odd halves for d (per chunk), h, w
_DORD = np.concatenate([
    np.concatenate([np.arange(d0, d0 + nd, 2), np.arange(d0 + 1, d0 + nd, 2)])
    for d0, nd in CHUNKS
])
_HORD = np.concatenate([np.arange(0, H, 2), np.arange(1, H, 2)])
_WORD = np.concatenate([np.arange(0, W, 2), np.arange(1, W, 2)])


def _make_cons(gamma: np.ndarray, beta: np.ndarray) -> np.ndarray:
    ga = gamma[0::2].astype(np.float64)
    go = gamma[1::2].astype(np.float64)
    grep = np.concatenate([ga, go])                      # raw, deinterleaved
    gw = (ga + go) / float(W)                            # gw'' = (ga+go)/64
    bw = 0.5 * (beta[0::2] + beta[1::2]).astype(np.float64)
    row1 = np.concatenate([gw, bw])
    return np.stack([grep, row1]).astype(NP_BF16)


def kernel(x, sum_weight, gamma, beta, trace=False):
    del sum_weight  # cancels exactly in LayerNorm (shift invariance)
    nc = _get_compiled()
    x = np.asarray(x)
    # permute d/h/w into even|odd halves, cast bf16
    xp = x[:, :, _DORD][:, :, :, _HORD][:, :, :, :, _WORD].astype(NP_BF16)
    cons = _make_cons(np.asarray(gamma), np.asarray(beta))
    in_maps = []
    for core in range(NCORES):
        shard = np.ascontiguousarray(
            xp[core * NPER : (core + 1) * NPER].reshape(P, D, H, W)
        )
        in_maps.append({"xs": shard, "cons": cons})
    res = run_bass_kernel_spmd(nc, in_maps, core_ids=list(range(NCORES)), trace=trace)
    out = np.concatenate(
        [
            res.results[i]["out"]
            .reshape(NPER, C, D // 2, H // 2, W // 2)
            .astype(np.float32)
            for i in range(NCORES)
        ],
        axis=0,
    )
    if trace:
        return out, res
    return out


if __name__ == "__main__":
    rng = np.random.default_rng(0)
    x = rng.standard_normal((N, C, D, H, W), dtype=np.float32)
    sw = rng.standard_normal((1,)).astype(np.float32)
    gamma = rng.random((W,), dtype=np.float32)
    beta = rng.standard_normal((W,)).astype(np.float32)
    y = kernel(x, sw, gamma, beta)
    print(y.shape, y.dtype)


# revision 31
# speedup vs baseline: 1.0857x; 1.0128x over previous
"""Trainium2 Bass kernel: x + s -> LayerNorm(W) -> 2x2x2 avgpool -> exact GELU.

Input  x: (32, 32, 16, 32, 64) f32, sum_weight (1,), gamma (64,), beta (64,)
Output:   (32, 32, 8, 16, 32) f32

Math:
  LN is shift-invariant, so sum_weight cancels exactly.
  pooled[q, w'] = sum_{r in quad} y_r (ga x_e + go x_o)[w'] - gw''[w'] mq[q] + bw[w']
    y_r   = rho_r / 8 = rsqrt(64 var_r)   (rsqrt via bit-magic + 1 Newton step)
    mq[q] = sum_{r in quad} r1_r y_r,  gw'' = (ga+go)/64,  bw = (be+bo)/2
  out = Gelu(pooled)

Layout: data-parallel over batch N (4 per core x 8 cores). Partitions = the
128 (n, c) pairs. Host pre-permutes d/h/w into even|odd halves and converts
to bf16, so every bulk op is a contiguous-half TENSOR_TENSOR that hits the
DVE 2x bf16 fast path (0.54 ns/elem measured on HW). Per-row stats come
from a 6-level pairwise add tree over a flat tile holding [x | x^2]; Square
and Gelu share one ACT table (one table load total). Everything else runs
on DVE: GPSIMD TT is ~2.6 ns/elem AND stalls concurrent DVE ops via shared
SBUF ports, so offloading there is net-negative. Chunks are variable-size
(small first/last) to cut pipeline fill and drain; issue order interleaves
A-phase of chunk k+2 with B-phase of chunk k.
"""

import numpy as np

import concourse.bacc as bacc
import concourse.bass as bass
import concourse.tile as tile
from concourse import mybir
from concourse.bass_utils import run_bass_kernel_spmd

P = 128
N, C, D, H, W = 32, 32, 16, 32, 64
NCORES = 8
NPER = N // NCORES
EPS = 1e-5
F32 = mybir.dt.float32
BF16 = mybir.dt.bfloat16
U32 = mybir.dt.uint32
NP_BF16 = mybir.dt.np(BF16)

# variable-size chunks (in d-planes): small first/last to cut pipeline
# fill and drain; middles large for low instruction overhead
CHUNKS = [(0, 2), (2, 4), (6, 4), (10, 4), (14, 2)]
HW2 = (H // 2) * 32   # 512: h' x w' per d' slot
MAGIC = 0x5F3759DF

AF = mybir.ActivationFunctionType
OP = mybir.AluOpType


def _kernel_body(ctx, tc: tile.TileContext, out_ap: bass.AP, xs: bass.AP, cons: bass.AP):
    nc = tc.nc

    singles = ctx.enter_context(tc.tile_pool(name="singles", bufs=1))
    xpool = ctx.enter_context(tc.tile_pool(name="xpool", bufs=2))
    tpool = ctx.enter_context(tc.tile_pool(name="tpool", bufs=1))
    t6pool = ctx.enter_context(tc.tile_pool(name="t6pool", bufs=2))
    gpool = ctx.enter_context(tc.tile_pool(name="gpool", bufs=2))
    spool = ctx.enter_context(tc.tile_pool(name="spool", bufs=2))
    small = ctx.enter_context(tc.tile_pool(name="small", bufs=1))
    opool = ctx.enter_context(tc.tile_pool(name="opool", bufs=2))

    xsf = xs.rearrange("p d h w -> p (d h w)")
    outf = out_ap.rearrange("p d h w -> p d (h w)")  # d' dim = 8 = NCHUNK*2

    state = {}
    stateB = {}
    prefetched = {}

    NDMAX = 4
    RMAX = NDMAX * H
    CMAX = RMAX * W
    NQMAX = (NDMAX // 2) * (H // 2)

    # issue chunk-0's data DMA before anything else (startup critical path)
    d0_0, nd_0 = CHUNKS[0]
    tX0 = xpool.tile([P, 2 * CMAX], BF16, tag="tX")
    nc.sync.dma_start(
        out=tX0[:, 0 : nd_0 * H * W],
        in_=xsf[:, d0_0 * H * W : (d0_0 + nd_0) * H * W],
    )
    prefetched[0] = tX0

    # constants (bf16), broadcast to all partitions
    grep_t = singles.tile([P, 64], BF16)   # [ga(32) | go(32)], raw gamma deint
    nc.sync.dma_start(out=grep_t[:], in_=cons[0:1, :].to_broadcast((P, 64)))
    gwbw_t = singles.tile([P, 64], BF16)   # [gw''(32) | bw(32)]
    nc.sync.dma_start(out=gwbw_t[:], in_=cons[1:2, :].to_broadcast((P, 64)))
    gw_t = gwbw_t[:, 0:32]
    bw_t = gwbw_t[:, 32:64]
    magic_t = singles.tile([P, 1], U32)
    nc.vector.memset(magic_t[:], MAGIC)

    def emit_A(k, t6p, off):
        d0, nd = CHUNKS[k]
        ROWS = nd * H
        CH = ROWS * W
        DC = nd
        NQ = (DC // 2) * (H // 2)
        # DMA + ACT square + DVE: gamma/wpool/tree/newton
        if k in prefetched:
            tXf = prefetched.pop(k)
        else:
            tXf = xpool.tile([P, 2 * CMAX], BF16, tag="tX")
            nc.sync.dma_start(
                out=tXf[:, 0:CH], in_=xsf[:, d0 * H * W : (d0 + nd) * H * W]
            )
        tx0 = tXf[:, 0:CH]
        tx1 = tXf[:, CH : 2 * CH]
        nc.scalar.activation(tx1, tx0, AF.Square)

        # g = x * gamma_rep ; s0 = g_lo + g_hi   (DVE bf16 2x)
        gf = gpool.tile([P, RMAX, W], BF16, tag="g")
        g = gf[:, 0:ROWS, :]
        nc.vector.tensor_tensor(
            out=g,
            in0=tx0.rearrange("p (r w) -> p r w", w=W),
            in1=grep_t[:].unsqueeze(1).to_broadcast((P, ROWS, W)),
            op=OP.mult,
        )
        s0f = spool.tile([P, RMAX, 32], BF16, tag="s0")
        s0 = s0f[:, 0:ROWS, :]
        nc.vector.tensor_tensor(
            out=s0, in0=g[:, :, 0:32], in1=g[:, :, 32:64], op=OP.add,
        )

        # stats tree: 5 levels of pairwise adds over [x | x^2], then L6
        # writes f32 sums into this pair's shared stats tile at `off`
        t_in = tXf[:, 0 : 2 * CH].rearrange("p (q w) -> p q w", w=W)
        for li, wd in enumerate((32, 16, 8, 4, 2)):
            tlf = tpool.tile([P, 2 * RMAX, wd], BF16, tag=f"tree{li}")
            tl = tlf[:, 0 : 2 * ROWS, :]
            nc.vector.tensor_tensor(
                out=tl, in0=t_in[:, :, 0:wd], in1=t_in[:, :, wd : 2 * wd],
                op=OP.add,
            )
            t_in = tl
        l5 = t_in.rearrange("p (t r) w -> p t r w", t=2)
        nc.vector.tensor_tensor(
            out=t6p[:, :, off : off + ROWS],
            in0=l5[:, :, :, 0],
            in1=l5[:, :, :, 1],
            op=OP.add,
        )

        state[k] = (s0, t6p, off, d0, nd)


    def emit_N(t6p, tot):
        # batched rsqrt(64*var) via bit magic + 1 Newton step over `tot` rows
        def sm(tag, dt=F32):
            return small.tile([P, 2 * RMAX], dt, tag=tag, name=tag)[:, 0:tot]

        r1 = t6p[:, 0, 0:tot]
        r2 = t6p[:, 1, 0:tot]
        msq = sm("msq")
        nc.vector.tensor_tensor(out=msq, in0=r1, in1=r1, op=OP.mult)
        wv2 = sm("wv2")
        nc.vector.scalar_tensor_tensor(
            out=wv2, in0=msq, scalar=-1.0 / W, in1=r2,
            op0=OP.mult, op1=OP.add,
        )
        yi = sm("yi", U32)
        nc.vector.tensor_scalar(
            out=yi, in0=wv2.bitcast(U32), scalar1=1, scalar2=None,
            op0=OP.logical_shift_right,
        )
        y0 = sm("y0", U32)
        nc.vector.tensor_tensor(
            out=y0, in0=magic_t[:].to_broadcast((P, tot)), in1=yi,
            op=OP.subtract,
        )
        ys = y0.bitcast(F32)
        a = sm("nta")
        nc.vector.tensor_tensor(out=a, in0=ys, in1=ys, op=OP.mult)
        b = sm("ntb")
        nc.vector.scalar_tensor_tensor(
            out=b, in0=a, scalar=-0.5, in1=wv2, op0=OP.mult, op1=OP.mult
        )
        yt = sm("nty")
        nc.vector.scalar_tensor_tensor(
            out=yt, in0=b, scalar=1.5, in1=ys, op0=OP.add, op1=OP.mult
        )
        mrs = sm("mrs")
        nc.vector.tensor_tensor(out=mrs, in0=r1, in1=yt, op=OP.mult)
        return yt, mrs

    def emit_M(k, yn):
        y_all, mrs_all = yn
        s0, t6p, off, d0, nd = state.pop(k)
        ROWS = nd * H
        DC = nd
        NQ = (DC // 2) * (H // 2)
        y = y_all[:, off : off + ROWS]
        srf = spool.tile([P, RMAX, 32], BF16, tag="sr")
        sr = srf[:, 0:ROWS, :]
        if k < len(CHUNKS) - 1:
            yrep = gpool.tile([P, RMAX, W], BF16, tag="g", name="yrep")
            nc.scalar.activation(
                yrep[:, 0:ROWS, 0:32],
                y.unsqueeze(2).to_broadcast((P, ROWS, 32)),
                AF.Copy,
            )
            nc.vector.tensor_tensor(
                out=sr, in0=s0, in1=yrep[:, 0:ROWS, 0:32], op=OP.mult,
            )
        else:
            nc.vector.tensor_tensor(
                out=sr, in0=s0,
                in1=y.unsqueeze(2).to_broadcast((P, ROWS, 32)), op=OP.mult,
            )
        sr4 = sr.rearrange("p (s h) w -> p s (h w)", s=DC)
        xdf = spool.tile([P, NDMAX // 2, H * 32], BF16, tag="xd")
        xd = xdf[:, 0 : DC // 2, :]
        nc.vector.tensor_tensor(
            out=xd, in0=sr4[:, 0 : DC // 2, :], in1=sr4[:, DC // 2 : DC, :],
            op=OP.add,
        )
        xhf = spool.tile([P, NDMAX // 2, HW2], BF16, tag="xh")
        xh = xhf[:, 0 : DC // 2, :]
        nc.vector.tensor_tensor(
            out=xh, in0=xd[:, :, 0:HW2], in1=xd[:, :, HW2 : 2 * HW2],
            op=OP.add,
        )
        mrs4 = mrs_all[:, off : off + ROWS].rearrange("p (s h) -> p s h", s=DC)
        m1 = small.tile([P, NDMAX // 2, H], F32, tag="m1", name="m1")[:, 0 : DC // 2, :]
        nc.vector.tensor_tensor(
            out=m1, in0=mrs4[:, 0 : DC // 2, :], in1=mrs4[:, DC // 2 : DC, :],
            op=OP.add,
        )
        mq = small.tile([P, NDMAX // 2, H // 2], F32, tag="mq", name="mq")[:, 0 : DC // 2, :]
        nc.vector.tensor_tensor(
            out=mq, in0=m1[:, :, 0 : H // 2], in1=m1[:, :, H // 2 : H],
            op=OP.add,
        )
        mq_b = mq.rearrange("p s h -> p (s h)").unsqueeze(2).to_broadcast((P, NQ, 32))
        corr = spool.tile([P, NQMAX, 32], BF16, tag="corr", name="corr")[:, 0:NQ, :]
        if k < len(CHUNKS) - 1:
            mqrep = opool.tile([P, NQMAX * 32], BF16, tag="pre", name="mqrep")
            mqr = mqrep[:, 0 : NQ * 32].rearrange("p (a b) -> p a b", b=32)
            nc.scalar.activation(mqr, mq_b, AF.Copy)
            nc.vector.tensor_tensor(
                out=corr, in0=gw_t.unsqueeze(1).to_broadcast((P, NQ, 32)),
                in1=mqr, op=OP.mult,
            )
        else:
            nc.vector.tensor_tensor(
                out=corr, in0=gw_t.unsqueeze(1).to_broadcast((P, NQ, 32)),
                in1=mq_b, op=OP.mult,
            )
        stateB[k] = (xh, corr, d0, nd)

    def emit_B(k):
        xh, corr, d0, nd = stateB.pop(k)
        NQ = (nd // 2) * (H // 2)
        pre = opool.tile([P, NQMAX * 32], BF16, tag="pre", name="pre")[:, 0 : NQ * 32]
        nc.vector.tensor_tensor(
            out=pre,
            in0=xh.rearrange("p a b -> p (a b)"),
            in1=corr.rearrange("p a b -> p (a b)"),
            op=OP.subtract,
        )
        pre2 = opool.tile([P, NQMAX, 32], BF16, tag="pre2", name="pre2")[:, 0:NQ, :]
        nc.vector.tensor_tensor(
            out=pre2,
            in0=pre.rearrange("p (a b) -> p a b", b=32),
            in1=bw_t.unsqueeze(1).to_broadcast((P, NQ, 32)),
            op=OP.add,
        )
        res = opool.tile([P, NQMAX * 32], BF16, tag="res", name="res")[:, 0 : NQ * 32]
        nc.scalar.activation(
            res, pre2.rearrange("p a b -> p (a b)"), AF.Gelu
        )
        nc.sync.dma_start(
            out=outf[:, d0 // 2 : d0 // 2 + nd // 2, :],
            in_=res.rearrange("p (a b) -> p a b", b=HW2),
        )

    # pairs share one stats tile and one batched newton pass
    # pipeline: A0 A1 N01 M0 A2 M1 B0 A3 N23 M2 B1 A4 M3 B2 N4 M4 B3 B4
    def pair_tile():
        t6p = t6pool.tile([P, 2, 2 * RMAX], F32, tag="t6p", name="t6p")
        return t6p

    r0 = CHUNKS[0][1] * H
    r1_ = CHUNKS[1][1] * H
    r2_ = CHUNKS[2][1] * H
    r3_ = CHUNKS[3][1] * H
    r4_ = CHUNKS[4][1] * H
    tp01 = pair_tile()
    emit_A(0, tp01, 0)
    emit_A(1, tp01, r0)
    y01 = emit_N(tp01, r0 + r1_)
    emit_M(0, y01)
    tp23 = pair_tile()
    emit_A(2, tp23, 0)
    emit_M(1, y01)
    emit_B(0)
    emit_A(3, tp23, r2_)
    y23 = emit_N(tp23, r2_ + r3_)
    emit_M(2, y23)
    emit_B(1)
    tp4 = pair_tile()
    emit_A(4, tp4, 0)
    emit_M(3, y23)
    emit_B(2)
    y4 = emit_N(tp4, r4_)
    emit_M(4, y4)
    emit_B(3)
    emit_B(4)


_CACHE: dict = {}


def _get_compiled():
    if "nc" not in _CACHE:
        nc = bacc.Bacc("TRN2", target_bir_lowering=False, debug=False)
        xs = nc.dram_tensor("xs", [P, D, H, W], BF16, kind="ExternalInput").ap()
        cons = nc.dram_tensor("cons", [2, 64], BF16, kind="ExternalInput").ap()
        out = nc.dram_tensor(
            "out", [P, D // 2, H // 2, W // 2], BF16, kind="ExternalOutput"
        ).ap()
        from contextlib import ExitStack

        with tile.TileContext(nc) as tc, ExitStack() as ctx:
            _kernel_body(ctx, tc, out, xs, cons)
        nc.compile()
        _CACHE["nc"] = nc
    return _CACHE["nc"]


# host-side index permutations: even|odd halves for d (per chunk), h, w
_DORD = np.concatenate([
    np.concatenate([np.arange(d0, d0 + nd, 2), np.arange(d0 + 1, d0 + nd, 2)])
    for d0, nd in CHUNKS
])
_HORD = np.concatenate([np.arange(0, H, 2), np.arange(1, H, 2)])
_WORD = np.concatenate([np.arange(0, W, 2), np.arange(1, W, 2)])


def _make_cons(gamma: np.ndarray, beta: np.ndarray) -> np.ndarray:
    ga = gamma[0::2].astype(np.float64)
    go = gamma[1::2].astype(np.float64)
    grep = np.concatenate([ga, go])                      # raw, deinterleaved
    gw = (ga + go) / float(W)                            # gw'' = (ga+go)/64
    bw = 0.5 * (beta[0::2] + beta[1::2]).astype(np.float64)
    row1 = np.concatenate([gw, bw])
    return np.stack([grep, row1]).astype(NP_BF16)


def kernel(x, sum_weight, gamma, beta, trace=False):
    del sum_weight  # cancels exactly in LayerNorm (shift invariance)
    nc = _get_compiled()
    x = np.asarray(x)
    # permute d/h/w into even|odd halves, cast bf16
    xp = x[:, :, _DORD][:, :, :, _HORD][:, :, :, :, _WORD].astype(NP_BF16)
    cons = _make_cons(np.asarray(gamma), np.asarray(beta))
    in_maps = []
    for core in range(NCORES):
        shard = np.ascontiguousarray(
            xp[core * NPER : (core + 1) * NPER].reshape(P, D, H, W)
        )
        in_maps.append({"xs": shard, "cons": cons})
    res = run_bass_kernel_spmd(nc, in_maps, core_ids=list(range(NCORES)), trace=trace)
    out = np.concatenate(
        [
            res.results[i]["out"]
            .reshape(NPER, C, D // 2, H // 2, W // 2)
            .astype(np.float32)
            for i in range(NCORES)
        ],
        axis=0,
    )
    if trace:
        return out, res
    return out


if __name__ == "__main__":
    rng = np.random.default_rng(0)
    x = rng.standard_normal((N, C, D, H, W), dtype=np.float32)
    sw = rng.standard_normal((1,)).astype(np.float32)
    gamma = rng.random((W,), dtype=np.float32)
    beta = rng.standard_normal((W,)).astype(np.float32)
    y = kernel(x, sw, gamma, beta)
    print(y.shape, y.dtype)
